# revision 2
# baseline (speedup 1.0000x reference)
"""Trainium2 Bass kernel for KronLinear:
    out = x @ (sum_r kron(a_r, b_r)) + bias
x: (8192, 4096) f32, a: (64,64,64), b: (64,64,64), bias: (4096,)

Sharding: 2-way over tokens x 4-way over output columns across 8 cores.
Each core:
  1. Builds its 1024-column slice of the Kronecker weight on device:
     Wtmp[(i,j),(k,l)] = sum_r a[r,i,j]*b[r,k,l] via PE matmuls (f32r),
     then fixes the (i,j,k,l) -> (i,k,j,l) layout with a strided
     DRAM round-trip (w scratch), keeping w resident in SBUF after.
  2. Streams host-pre-tiled x^T tiles and accumulates
     out[m, n] = sum_K xT[K, m] * w[K, n] over 32 K-tiles into PSUM
     (f32r matmuls, N=512), adds bias on DVE, DMAs out.
"""
import numpy as np

RANK = 64
A1 = A2 = B1 = B2 = 64
NTOK = 8192
NCORES = 8
TH = 2            # token shards
CQ = 4            # column shards
TOK_SH = NTOK // TH          # 4096 tokens per core
COLS_SH = (A2 * B2) // CQ    # 1024 out cols per core
JPC = A2 // CQ               # 16 j-values per core
MT = TOK_SH // 128           # 32 m-tiles
KT = (A1 * B1) // 128        # 32 k-tiles

_CACHE = {}


def _build_nc(debug=False):
    import sys
    if "/opt/trn_rl_repo" not in sys.path:
        sys.path.insert(0, "/opt/trn_rl_repo")
    import concourse.tile as tile
    from concourse import bacc, mybir

    f32 = mybir.dt.float32
    f32r = mybir.dt.float32r

    nc = bacc.Bacc(None, target_bir_lowering=False, debug=debug,
                   num_devices=NCORES)

    xt_d = nc.dram_tensor("xt", [MT, 128, KT * 128], f32r, kind="ExternalInput")
    asel_d = nc.dram_tensor("asel", [RANK, A1 * JPC], f32r, kind="ExternalInput")
    b_d = nc.dram_tensor("bb", [RANK, B1 * B2], f32r, kind="ExternalInput")
    bias_d = nc.dram_tensor("bias", [1, COLS_SH], f32, kind="ExternalInput")
    out_d = nc.dram_tensor("out", [TOK_SH, COLS_SH], f32, kind="ExternalOutput")

    with tile.TileContext(nc) as tc:
        with tc.tile_pool(name="dram", bufs=1, space="DRAM") as dpool, \
             tc.tile_pool(name="const", bufs=1) as cpool, \
             tc.tile_pool(name="stg", bufs=3) as spool, \
             tc.tile_pool(name="wres", bufs=1) as wpool, \
             tc.tile_pool(name="xin", bufs=2) as xpool, \
             tc.tile_pool(name="oout", bufs=2) as opool, \
             tc.tile_pool(name="wps", bufs=2, space="PSUM") as wps_pool, \
             tc.tile_pool(name="mps", bufs=2, space="PSUM") as mps_pool:

            w_t = dpool.tile([A1 * B1, COLS_SH], f32r)  # w scratch in DRAM

            asel = cpool.tile([RANK, A1 * JPC], f32r)
            b2d = cpool.tile([RANK, B1 * B2], f32r)
            bias_sb = cpool.tile([128, COLS_SH], f32)
            nc.sync.dma_start(out=asel[:], in_=asel_d[:, :])
            nc.sync.dma_start(out=b2d[:], in_=b_d[:, :])
            nc.sync.dma_start(
                out=bias_sb[:],
                in_=bias_d[:, :].broadcast_to([128, COLS_SH]))

            # ---- Prologue: build w slice, layout-fixed, into DRAM scratch
            asel3 = asel[:, :].rearrange("r (i j) -> r i j", i=A1)
            for ib in range(8):          # i-blocks of 8
                for kt8 in range(8):     # (k,l)-tiles of 512
                    ps = wps_pool.tile([128, 512], f32)
                    nc.tensor.matmul(
                        ps[:],
                        asel3[:, ib * 8:(ib + 1) * 8, :],
                        b2d[:, kt8 * 512:(kt8 + 1) * 512],
                        start=True, stop=True)
                    stg = spool.tile([128, 512], f32r)
                    nc.vector.tensor_copy(stg[:], ps[:])
                    # scatter into w: row i*64 + k, col j*64 + l
                    for irel in range(8):
                        i = ib * 8 + irel
                        src_v = stg[irel * JPC:(irel + 1) * JPC, :] \
                            .rearrange("j (k l) -> j k l", k=8)
                        dst_v = w_t[i * 64 + kt8 * 8: i * 64 + kt8 * 8 + 8, :] \
                            .rearrange("k (j l) -> k j l", j=JPC) \
                            .transpose([1, 0, 2])
                        nc.sync.dma_start(out=dst_v, in_=src_v)

            # ---- Read w back, resident in SBUF
            w_sb = []
            for kt in range(KT):
                wt = wpool.tile([128, COLS_SH], f32r, tag=f"w{kt}")
                nc.sync.dma_start(out=wt[:], in_=w_t[kt * 128:(kt + 1) * 128, :])
                w_sb.append(wt)

            # ---- Main loop over token tiles
            for mt in range(MT):
                xts = xpool.tile([128, KT * 128], f32r)
                nc.sync.dma_start(out=xts[:], in_=xt_d[mt, :, :])
                ps = mps_pool.tile([128, COLS_SH], f32)
                for kt in range(KT):
                    lt = xts[:, kt * 128:(kt + 1) * 128]
                    nc.tensor.matmul(ps[:, 0:512], lt,
                                     w_sb[kt][:, 0:512],
                                     start=(kt == 0), stop=(kt == KT - 1))
                    nc.tensor.matmul(ps[:, 512:1024], lt,
                                     w_sb[kt][:, 512:1024],
                                     start=(kt == 0), stop=(kt == KT - 1))
                osb = opool.tile([128, COLS_SH], f32)
                nc.vector.tensor_add(osb[:], ps[:], bias_sb[:])
                nc.sync.dma_start(out=out_d[mt * 128:(mt + 1) * 128, :],
                                  in_=osb[:])

    nc.compile()
    return nc


def _host_prep(x, a, b, bias):
    """Build per-core input maps."""
    x = np.ascontiguousarray(x, dtype=np.float32)
    a = np.asarray(a, dtype=np.float32)
    b = np.asarray(b, dtype=np.float32)
    bias = np.asarray(bias, dtype=np.float32)

    b2d = np.ascontiguousarray(b.reshape(RANK, B1 * B2))
    xt_by_th = []
    for th in range(TH):
        xh = x[th * TOK_SH:(th + 1) * TOK_SH]
        # A[mt, p, kt, mm] = x[mt*128+mm, kt*128+p]
        x4 = xh.reshape(MT, 128, KT, 128)
        xt = np.ascontiguousarray(x4.transpose(0, 3, 2, 1)).reshape(MT, 128, KT * 128)
        xt_by_th.append(xt)
    asel_by_cq = []
    bias_by_cq = []
    for cq in range(CQ):
        asel = np.ascontiguousarray(
            a[:, :, cq * JPC:(cq + 1) * JPC].reshape(RANK, A1 * JPC))
        asel_by_cq.append(asel)
        bias_by_cq.append(np.ascontiguousarray(
            bias[cq * COLS_SH:(cq + 1) * COLS_SH].reshape(1, COLS_SH)))

    in_maps = []
    for c in range(NCORES):
        th, cq = c // CQ, c % CQ
        in_maps.append({
            "xt": xt_by_th[th],
            "asel": asel_by_cq[cq],
            "bb": b2d,
            "bias": bias_by_cq[cq],
        })
    return in_maps


def kernel(x, a, b, bias):
    import sys
    if "/opt/trn_rl_repo" not in sys.path:
        sys.path.insert(0, "/opt/trn_rl_repo")
    from concourse.bass_utils import run_bass_kernel_spmd

    if "nc" not in _CACHE:
        _CACHE["nc"] = _build_nc(debug=False)
    nc = _CACHE["nc"]

    in_maps = _host_prep(x, a, b, bias)
    res = run_bass_kernel_spmd(nc, in_maps, core_ids=list(range(NCORES)))
    out = np.empty((NTOK, A2 * B2), dtype=np.float32)
    for c in range(NCORES):
        th, cq = c // CQ, c % CQ
        out[th * TOK_SH:(th + 1) * TOK_SH,
            cq * COLS_SH:(cq + 1) * COLS_SH] = res.results[c]["out"]
    return out
